# revision 31
# baseline (speedup 1.0000x reference)
"""Single-head attention Bass kernel for 8 TRN2 cores — v3: pair-deduped K/V.

Cores pair up per batch element (2b, 2b+1). Each core projects K/V only for
its OWN query half (host-rotated to columns 0:NQ of its x input), then the
pair exchanges shards with four pipelined 2-rank AllGathers (K in two 512-col
chunks, V in two 4-tile chunks) while the PE computes the Q projection and
the early attention tiles.

Key trick: kT/vN hold the keys in GATHER order ([rank0 | rank1] per chunk),
which is the same fixed layout on both cores — softmax over keys is
order-invariant, so no per-core indexing is needed anywhere. The local
shard regions double as staging scratch before the exchange.
"""
import sys
import numpy as np

for p in ("/opt/trn_rl_repo",):
    if p not in sys.path:
        sys.path.insert(0, p)

import ml_dtypes

B, S, D = 4, 2048, 1024
NQ = 1024
NCORES = 8
P = 128
INV_SQRT_D = 1.0 / 32.0
BF16 = ml_dtypes.bfloat16

_CACHE = {}


def build_nc():
    from contextlib import ExitStack
    import concourse.mybir as mybir
    import concourse.tile as tile
    from concourse import bacc

    F32 = mybir.dt.float32
    BF = mybir.dt.bfloat16
    AF = mybir.ActivationFunctionType
    GROUPS = [[0, 1], [2, 3], [4, 5], [6, 7]]

    nc = bacc.Bacc("TRN2", debug=False)

    xlT = nc.dram_tensor("xlT", (D, NQ), BF, kind="ExternalInput")
    wqT = nc.dram_tensor("wqT", (D, D), BF, kind="ExternalInput")
    wkT = nc.dram_tensor("wkT", (D, D), BF, kind="ExternalInput")
    wvT = nc.dram_tensor("wvT", (D, D), BF, kind="ExternalInput")
    bqk = nc.dram_tensor("bqk", (P, 16), F32, kind="ExternalInput")
    bv = nc.dram_tensor("bv", (D,), F32, kind="ExternalInput")
    out = nc.dram_tensor("out", (NQ, D), F32, kind="ExternalOutput")

    ET = D // P
    DT = D // P
    LC = NQ // 512         # 2 local x chunks
    SB = S // P            # 16 j-tiles in gather order
    HB = SB // 2
    IG = NQ // 512
    EC = D // 512

    with tile.TileContext(nc) as tc, ExitStack() as ctx:
        consts = ctx.enter_context(tc.tile_pool(name="consts", bufs=1))
        wpool = ctx.enter_context(tc.tile_pool(name="wpool", bufs=1))
        xlpool = ctx.enter_context(tc.tile_pool(name="xlpool", bufs=1))
        qpool = ctx.enter_context(tc.tile_pool(name="qpool", bufs=1))
        kpool = ctx.enter_context(tc.tile_pool(name="kpool", bufs=1))
        vpool = ctx.enter_context(tc.tile_pool(name="vpool", bufs=1))
        dram = ctx.enter_context(tc.tile_pool(name="dram", bufs=1, space="DRAM"))

        wq_sb = wpool.tile([P, DT, D], BF)
        wk_sb = wpool.tile([P, DT, D], BF)
        wv_sb = wpool.tile([P, DT, D], BF)
        xl_sb = xlpool.tile([P, DT, NQ], BF)
        qT = qpool.tile([P, ET, NQ], BF)
        # kT columns / vN tiles live in GATHER order: per 512-chunk c of the
        # local projection, gathered cols [2c*512 : (2c+2)*512] = [r0 | r1]
        kT = kpool.tile([P, ET, S], BF)
        vN = vpool.tile([P, SB, D], BF)

        cc_ink = [dram.tile([P, ET, 512], BF, name=f"cc_ink{c}") for c in range(LC)]
        cc_outk = [dram.tile([2, P, ET, 512], BF, name=f"cc_outk{c}") for c in range(LC)]
        cc_inv = [dram.tile([P, 4, D], BF, name=f"cc_inv{c}") for c in range(LC)]
        cc_outv = [dram.tile([2, P, 4, D], BF, name=f"cc_outv{c}") for c in range(LC)]

        # ---- startup DMAs: K inputs first (wk + xl), then wv, then wq ----
        _q = (nc.sync, nc.gpsimd, nc.scalar)
        _i = [0]
        def dma_rr(dst, src):
            _q[_i[0] % 3].dma_start(dst, src)
            _i[0] += 1

        dma_rr(wk_sb[:, 0, 0:512], wkT[0:P, 0:512])
        dma_rr(xl_sb[:, 0, 0:512], xlT[0:P, 0:512])
        dma_rr(wk_sb[:, 0, 512:D], wkT[0:P, 512:D])
        for dt in range(1, DT):
            dma_rr(wk_sb[:, dt, :], wkT[dt * P:(dt + 1) * P, :])
            dma_rr(xl_sb[:, dt, 0:512], xlT[dt * P:(dt + 1) * P, 0:512])
        for dt in range(DT):
            dma_rr(xl_sb[:, dt, 512:NQ], xlT[dt * P:(dt + 1) * P, 512:NQ])
        for dt in range(DT):
            dma_rr(wv_sb[:, dt, :], wvT[dt * P:(dt + 1) * P, :])
        for dt in range(DT):
            dma_rr(wq_sb[:, dt, :], wqT[dt * P:(dt + 1) * P, :])

        bqk_sb = consts.tile([P, 16], F32)
        nc.scalar.dma_start(bqk_sb[:], bqk[:, :])
        bv_f = consts.tile([1, D], F32)
        nc.scalar.dma_start(bv_f[:], bv[:].rearrange("(one d) -> one d", one=1))
        ones_f = consts.tile([1, P], F32)
        nc.gpsimd.memset(ones_f[:], 1.0)
        ones_row = consts.tile([1, P], BF)
        nc.vector.tensor_copy(ones_row[:], ones_f[:])
        onesc_f = consts.tile([P, 1], F32)
        nc.gpsimd.memset(onesc_f[:], 1.0)
        ones_col = consts.tile([P, 1], BF)
        nc.vector.tensor_copy(ones_col[:], onesc_f[:])
        bv_bf = consts.tile([1, D], BF)
        nc.vector.tensor_copy(bv_bf[:], bv_f[:])
        bv_bcast = consts.tile([P, D], F32)

        with tc.tile_pool(name="bigps", bufs=1, space="PSUM") as bigps:
            def banks(prefix):
                return [bigps.tile([P, 512], F32, tag=f"b{i}", name=f"{prefix}{i}")
                        for i in range(8)]

            # ---- Phase A: local K projection chunks; each chunk is staged
            # into kT scratch, bounced out, AllGathered, and read back in
            # gather order ----
            for lc in range(LC):
                c0 = lc * 512
                # scratch region: must only be covered by THIS chunk's own
                # readback, else the next chunk's writes would wait on the
                # previous collective (lc0 -> cols 0:512, lc1 -> 1536:2048)
                s0 = 0 if lc == 0 else S - 512
                psks = banks(f"psk{lc}_")
                # dt-outer both chunks: consume x rows as the DMAs land
                for dt in range(DT):
                    for et in range(ET):
                        nc.tensor.matmul(psks[et][:], wk_sb[:, dt, et * P:(et + 1) * P],
                                         xl_sb[:, dt, c0:c0 + 512],
                                         start=(dt == 0), stop=(dt == DT - 1))
                for et in range(ET):
                    nc.vector.tensor_scalar_add(kT[:, et, s0:s0 + 512], psks[et][:],
                                                bqk_sb[:, ET + et:ET + et + 1])
                # bounce on sync+scalar only: gpsimd must stay free so the
                # collective trigger fires as soon as staging completes
                nc.sync.dma_start(cc_ink[lc][:, 0:4, :], kT[:, 0:4, s0:s0 + 512])
                nc.scalar.dma_start(cc_ink[lc][:, 4:8, :], kT[:, 4:8, s0:s0 + 512])
                nc.gpsimd.collective_compute(
                    "AllGather", mybir.AluOpType.bypass, replica_groups=GROUPS,
                    ins=[cc_ink[lc][:]], outs=[cc_outk[lc][:]],
                )
                # gathered chunk -> kT cols [2c0 : 2c0+1024], one DMA per
                # rank slot (AP balancer caps at 3 dims)
                for r in range(2):
                    e = (nc.scalar, nc.sync)[r]
                    e.dma_start(kT[:, :, 2 * c0 + r * 512:2 * c0 + (r + 1) * 512],
                                cc_outk[lc][r])

            # ---- Phase B: local V projection chunks, same exchange ----
            for lc in range(LC):
                c0 = lc * 512
                # scratch tiles covered only by this chunk's own readback
                # (lc0 -> tiles 0:4, lc1 -> tiles 12:16)
                t0 = 0 if lc == 0 else SB - 4
                psvs = banks(f"psv{lc}_")
                if lc == 0:
                    for dt in range(DT):
                        for j4 in range(4):
                            for ec in range(EC):
                                nc.tensor.matmul(psvs[j4 * 2 + ec][:],
                                                 xl_sb[:, dt, c0 + j4 * P:c0 + (j4 + 1) * P],
                                                 wv_sb[:, dt, ec * 512:(ec + 1) * 512],
                                                 start=(dt == 0), stop=(dt == DT - 1))
                    for j4 in range(4):
                        for ec in range(EC):
                            nc.any.tensor_copy(vN[:, t0 + j4, ec * 512:(ec + 1) * 512],
                                               psvs[j4 * 2 + ec][:])
                else:
                    for j4 in range(4):
                        for ec in range(EC):
                            for dt in range(DT):
                                nc.tensor.matmul(psvs[j4 * 2 + ec][:],
                                                 xl_sb[:, dt, c0 + j4 * P:c0 + (j4 + 1) * P],
                                                 wv_sb[:, dt, ec * 512:(ec + 1) * 512],
                                                 start=(dt == 0), stop=(dt == DT - 1))
                            nc.any.tensor_copy(vN[:, t0 + j4, ec * 512:(ec + 1) * 512],
                                               psvs[j4 * 2 + ec][:])
                nc.sync.dma_start(cc_inv[lc][:, 0:2, :], vN[:, t0:t0 + 2, :])
                nc.scalar.dma_start(cc_inv[lc][:, 2:4, :], vN[:, t0 + 2:t0 + 4, :])
                nc.gpsimd.collective_compute(
                    "AllGather", mybir.AluOpType.bypass, replica_groups=GROUPS,
                    ins=[cc_inv[lc][:]], outs=[cc_outv[lc][:]],
                )
                for r in range(2):
                    e = (nc.scalar, nc.sync)[r]
                    e.dma_start(vN[:, 8 * lc + 4 * r:8 * lc + 4 * (r + 1), :],
                                cc_outv[lc][r])

            # ---- Phase C: Q projection group 0 only (group 1 is deferred
            # into phase D to push the AG-K2-dependent scores later) ----
            for g in range(1):
                psqs = banks(f"psq{g}_")
                for et in range(ET):
                    for dt in range(DT):
                        nc.tensor.matmul(psqs[et][:], wq_sb[:, dt, et * P:(et + 1) * P],
                                         xl_sb[:, dt, g * 512:(g + 1) * 512],
                                         start=(dt == 0), stop=(dt == DT - 1))
                    nc.vector.tensor_scalar_add(qT[:, et, g * 512:(g + 1) * 512],
                                                psqs[et][:], bqk_sb[:, et:et + 1])
            for ec in range(EC):
                pstmp = bigps.tile([P, 512], F32, tag=f"b{ec}", name=f"pstmp{ec}")
                nc.tensor.matmul(pstmp[:], ones_row[:], bv_bf[:, ec * 512:(ec + 1) * 512],
                                 start=True, stop=True)
                nc.any.tensor_copy(bv_bcast[:, ec * 512:(ec + 1) * 512], pstmp[:])

        # ---- Phase D: attention over gather-ordered tiles ----
        with tc.tile_pool(name="sps", bufs=2, space="PSUM") as ps512, \
             tc.tile_pool(name="outps", bufs=2, space="PSUM") as outps, \
             tc.tile_pool(name="rsps", bufs=2, space="PSUM") as rsps, \
             tc.tile_pool(name="attn", bufs=2) as attnp, \
             tc.tile_pool(name="epi", bufs=2) as epip:
            aTs = {}
            for g in range(IG):
                aTs[g] = attnp.tile([P, SB, 512], BF, tag="attn", name=f"aT{g}")
            def scores_tile(g, jt):
                sps = ps512.tile([P, 512], F32, tag="ps512", name="sps")
                for et in range(ET):
                    nc.tensor.matmul(sps[:], kT[:, et, jt * P:(jt + 1) * P],
                                     qT[:, et, g * 512:(g + 1) * 512],
                                     start=(et == 0), stop=(et == ET - 1))
                nc.scalar.activation(aTs[g][:, jt, :], sps[:], AF.Exp,
                                     scale=INV_SQRT_D)
            # AG-K1-dependent tiles first, with the deferred Q-projection
            # of group 1 wedged in between so the AG-K2-dependent tiles
            # start as late as possible (absorbing collective latency)
            for jt in range(HB):
                scores_tile(0, jt)
            for et in range(ET):
                psq = ps512.tile([P, 512], F32, tag="ps512", name="psq1")
                for dt in range(DT):
                    nc.tensor.matmul(psq[:], wq_sb[:, dt, et * P:(et + 1) * P],
                                     xl_sb[:, dt, 512:NQ],
                                     start=(dt == 0), stop=(dt == DT - 1))
                nc.vector.tensor_scalar_add(qT[:, et, 512:NQ],
                                            psq[:], bqk_sb[:, et:et + 1])
            for jt in range(HB):
                scores_tile(1, jt)
            for g in range(IG):
                for jt in range(HB, SB):
                    scores_tile(g, jt)

            for g in range(IG):
                aT = aTs[g]
                invT = epip.tile([P, 8], F32, tag="invT")
                for ib in range(4):
                    rs_ps = rsps.tile([P, 2], F32, tag="rs", name=f"rs{g}_{ib}")
                    last = (g == IG - 1 and ib == 3)
                    out_ps = [outps.tile([P, 512], F32, tag=f"outps{ec}", name=f"out_ps{ec}")
                              for ec in range(EC)]
                    if not last:
                        for jg in range(SB):
                            for ec in range(EC):
                                nc.tensor.matmul(out_ps[ec][:],
                                                 aT[:, jg, ib * P:(ib + 1) * P],
                                                 vN[:, jg, ec * 512:(ec + 1) * 512],
                                                 start=(jg == 0), stop=(jg == SB - 1))
                            nc.tensor.matmul(rs_ps[:, 0:1],
                                             aT[:, jg, ib * P:(ib + 1) * P],
                                             ones_col[:],
                                             start=(jg == 0), stop=(jg == SB - 1))
                    else:
                        for jg in range(SB):
                            nc.tensor.matmul(rs_ps[:, 0:1],
                                             aT[:, jg, ib * P:(ib + 1) * P],
                                             ones_col[:],
                                             start=(jg == 0), stop=(jg == SB - 1))
                        for ec in range(EC):
                            for jg in range(SB):
                                nc.tensor.matmul(out_ps[ec][:],
                                                 aT[:, jg, ib * P:(ib + 1) * P],
                                                 vN[:, jg, ec * 512:(ec + 1) * 512],
                                                 start=(jg == 0), stop=(jg == SB - 1))
                    nc.vector.reciprocal(invT[:, 2 * ib:2 * ib + 1],
                                         rs_ps[:, 0:1])
                    out_sb = epip.tile([P, D], F32, tag="out_sb")
                    r0 = g * 512 + ib * P
                    if not last:
                        for ec in range(EC):
                            nc.vector.tensor_scalar_mul(out_sb[:, ec * 512:(ec + 1) * 512],
                                                        out_ps[ec][:], invT[:, 2 * ib:2 * ib + 1])
                        nc.vector.tensor_add(out_sb[:], out_sb[:], bv_bcast[:])
                        e = (nc.sync, nc.gpsimd, nc.scalar)[ib % 3]
                        e.dma_start(out[r0:r0 + P, :], out_sb[:])
                    else:
                        for ec in range(EC):
                            nc.vector.tensor_scalar_mul(out_sb[:, ec * 512:(ec + 1) * 512],
                                                        out_ps[ec][:], invT[:, 2 * ib:2 * ib + 1])
                            nc.vector.tensor_add(out_sb[:, ec * 512:(ec + 1) * 512],
                                                 out_sb[:, ec * 512:(ec + 1) * 512],
                                                 bv_bcast[:, ec * 512:(ec + 1) * 512])
                            e = (nc.sync, nc.gpsimd)[ec]
                            e.dma_start(out[r0:r0 + P, ec * 512:(ec + 1) * 512],
                                        out_sb[:, ec * 512:(ec + 1) * 512])

    nc.compile()
    return nc


def make_in_maps(x, Wq, bq, Wk, bk, Wv, bv):
    x = np.asarray(x, np.float32)
    wqT = np.asarray(Wq, np.float32).T.astype(BF16)
    wkT = np.asarray(Wk, np.float32).T.astype(BF16)
    wvT = np.asarray(Wv, np.float32).T.astype(BF16)
    bq = np.asarray(bq, np.float32)
    bk = np.asarray(bk, np.float32)
    bqk = np.ascontiguousarray(
        np.concatenate([bq.reshape(8, P).T, bk.reshape(8, P).T], axis=1)
    ).astype(np.float32)
    bv = np.ascontiguousarray(np.asarray(bv, np.float32))
    in_maps = []
    for c in range(NCORES):
        b, h = c // 2, c % 2
        xb = x[b]
        in_maps.append({
            "xlT": xb[h * NQ:(h + 1) * NQ].T.astype(BF16),
            "wqT": wqT, "wkT": wkT, "wvT": wvT,
            "bqk": bqk, "bv": bv,
        })
    return in_maps


def get_nc():
    if "nc" not in _CACHE:
        _CACHE["nc"] = build_nc()
    return _CACHE["nc"]


def kernel(x, Wq, bq, Wk, bk, Wv, bv):
    from concourse.bass_utils import run_bass_kernel_spmd
    nc = get_nc()
    in_maps = make_in_maps(x, Wq, bq, Wk, bk, Wv, bv)
    res = run_bass_kernel_spmd(nc, in_maps, core_ids=list(range(NCORES)))
    out = np.empty((B, S, D), np.float32)
    for c in range(NCORES):
        b, h = c // 2, c % 2
        out[b, h * NQ:(h + 1) * NQ] = res.results[c]["out"]
    return out


# revision 32
# speedup vs baseline: 1.1505x; 1.1505x over previous
"""Single-head attention Bass kernel for 8 TRN2 cores — v3: pair-deduped K/V.

Cores pair up per batch element (2b, 2b+1). Each core projects K/V only for
its OWN query half (host-rotated to columns 0:NQ of its x input), then the
pair exchanges shards with four pipelined 2-rank AllGathers (K in two 512-col
chunks, V in two 4-tile chunks) while the PE computes the Q projection and
the early attention tiles.

Key trick: kT/vN hold the keys in GATHER order ([rank0 | rank1] per chunk),
which is the same fixed layout on both cores — softmax over keys is
order-invariant, so no per-core indexing is needed anywhere. The local
shard regions double as staging scratch before the exchange.
"""
import sys
import numpy as np

for p in ("/opt/trn_rl_repo",):
    if p not in sys.path:
        sys.path.insert(0, p)

import ml_dtypes

B, S, D = 4, 2048, 1024
NQ = 1024
NCORES = 8
P = 128
INV_SQRT_D = 1.0 / 32.0
BF16 = ml_dtypes.bfloat16

_CACHE = {}


def build_nc():
    from contextlib import ExitStack
    import concourse.mybir as mybir
    import concourse.tile as tile
    from concourse import bacc

    F32 = mybir.dt.float32
    BF = mybir.dt.bfloat16
    AF = mybir.ActivationFunctionType
    GROUPS = [[0, 1], [2, 3], [4, 5], [6, 7]]

    nc = bacc.Bacc("TRN2", debug=False)

    xlT = nc.dram_tensor("xlT", (D, NQ), BF, kind="ExternalInput")
    wqT = nc.dram_tensor("wqT", (D, D), BF, kind="ExternalInput")
    wkT = nc.dram_tensor("wkT", (D, D), BF, kind="ExternalInput")
    wvT = nc.dram_tensor("wvT", (D, D), BF, kind="ExternalInput")
    bqk = nc.dram_tensor("bqk", (P, 16), F32, kind="ExternalInput")
    bv = nc.dram_tensor("bv", (D,), F32, kind="ExternalInput")
    out = nc.dram_tensor("out", (NQ, D), F32, kind="ExternalOutput")

    ET = D // P
    DT = D // P
    LC = NQ // 512         # 2 local x chunks
    SB = S // P            # 16 j-tiles in gather order
    HB = SB // 2
    IG = NQ // 512
    EC = D // 512

    with tile.TileContext(nc) as tc, ExitStack() as ctx:
        consts = ctx.enter_context(tc.tile_pool(name="consts", bufs=1))
        wpool = ctx.enter_context(tc.tile_pool(name="wpool", bufs=1))
        xlpool = ctx.enter_context(tc.tile_pool(name="xlpool", bufs=1))
        qpool = ctx.enter_context(tc.tile_pool(name="qpool", bufs=1))
        kpool = ctx.enter_context(tc.tile_pool(name="kpool", bufs=1))
        vpool = ctx.enter_context(tc.tile_pool(name="vpool", bufs=1))
        dram = ctx.enter_context(tc.tile_pool(name="dram", bufs=1, space="DRAM"))

        wq_sb = wpool.tile([P, DT, D], BF)
        wk_sb = wpool.tile([P, DT, D], BF)
        wv_sb = wpool.tile([P, DT, D], BF)
        xl_sb = xlpool.tile([P, DT, NQ], BF)
        qT = qpool.tile([P, ET, NQ], BF)
        # kT columns / vN tiles live in GATHER order: per 512-chunk c of the
        # local projection, gathered cols [2c*512 : (2c+2)*512] = [r0 | r1]
        kT = kpool.tile([P, ET, S], BF)
        vN = vpool.tile([P, SB, D], BF)

        cc_ink = [dram.tile([P, ET, 512], BF, name=f"cc_ink{c}") for c in range(LC)]
        cc_outk = [dram.tile([2, P, ET, 512], BF, name=f"cc_outk{c}") for c in range(LC)]
        cc_inv = [dram.tile([P, 4, D], BF, name=f"cc_inv{c}") for c in range(LC)]
        cc_outv = [dram.tile([2, P, 4, D], BF, name=f"cc_outv{c}") for c in range(LC)]

        # ---- startup DMAs: K inputs first (wk + xl), then wv, then wq ----
        _q = (nc.sync, nc.gpsimd, nc.scalar)
        _i = [0]
        def dma_rr(dst, src):
            _q[_i[0] % 3].dma_start(dst, src)
            _i[0] += 1

        dma_rr(wk_sb[:, 0, 0:512], wkT[0:P, 0:512])
        dma_rr(xl_sb[:, 0, 0:512], xlT[0:P, 0:512])
        dma_rr(wk_sb[:, 0, 512:D], wkT[0:P, 512:D])
        for dt in range(1, DT):
            dma_rr(wk_sb[:, dt, :], wkT[dt * P:(dt + 1) * P, :])
            dma_rr(xl_sb[:, dt, 0:512], xlT[dt * P:(dt + 1) * P, 0:512])
        for dt in range(DT):
            dma_rr(xl_sb[:, dt, 512:NQ], xlT[dt * P:(dt + 1) * P, 512:NQ])
        for dt in range(DT):
            dma_rr(wv_sb[:, dt, :], wvT[dt * P:(dt + 1) * P, :])
        for dt in range(DT):
            dma_rr(wq_sb[:, dt, :], wqT[dt * P:(dt + 1) * P, :])

        bqk_sb = consts.tile([P, 16], F32)
        nc.scalar.dma_start(bqk_sb[:], bqk[:, :])
        bv_f = consts.tile([1, D], F32)
        nc.scalar.dma_start(bv_f[:], bv[:].rearrange("(one d) -> one d", one=1))
        ones_f = consts.tile([1, P], F32)
        nc.gpsimd.memset(ones_f[:], 1.0)
        ones_row = consts.tile([1, P], BF)
        nc.vector.tensor_copy(ones_row[:], ones_f[:])
        onesc_f = consts.tile([P, 1], F32)
        nc.gpsimd.memset(onesc_f[:], 1.0)
        ones_col = consts.tile([P, 1], BF)
        nc.vector.tensor_copy(ones_col[:], onesc_f[:])
        bv_bf = consts.tile([1, D], BF)
        nc.vector.tensor_copy(bv_bf[:], bv_f[:])
        bv_bcast = consts.tile([P, D], F32)

        with tc.tile_pool(name="bigps", bufs=1, space="PSUM") as bigps:
            def banks(prefix):
                return [bigps.tile([P, 512], F32, tag=f"b{i}", name=f"{prefix}{i}")
                        for i in range(8)]

            # ---- Phase A: local K projection chunks; each chunk is staged
            # into kT scratch, bounced out, AllGathered, and read back in
            # gather order ----
            for lc in range(LC):
                c0 = lc * 512
                # scratch region: must only be covered by THIS chunk's own
                # readback, else the next chunk's writes would wait on the
                # previous collective (lc0 -> cols 0:512, lc1 -> 1536:2048)
                s0 = 0 if lc == 0 else S - 512
                psks = banks(f"psk{lc}_")
                # dt-outer both chunks: consume x rows as the DMAs land
                for dt in range(DT):
                    for et in range(ET):
                        nc.tensor.matmul(psks[et][:], wk_sb[:, dt, et * P:(et + 1) * P],
                                         xl_sb[:, dt, c0:c0 + 512],
                                         start=(dt == 0), stop=(dt == DT - 1))
                for et in range(ET):
                    nc.vector.tensor_scalar_add(kT[:, et, s0:s0 + 512], psks[et][:],
                                                bqk_sb[:, ET + et:ET + et + 1])
                # bounce on sync+scalar only: gpsimd must stay free so the
                # collective trigger fires as soon as staging completes
                nc.sync.dma_start(cc_ink[lc][:, 0:4, :], kT[:, 0:4, s0:s0 + 512])
                nc.scalar.dma_start(cc_ink[lc][:, 4:8, :], kT[:, 4:8, s0:s0 + 512])
                nc.gpsimd.collective_compute(
                    "AllGather", mybir.AluOpType.bypass, replica_groups=GROUPS,
                    ins=[cc_ink[lc][:]], outs=[cc_outk[lc][:]],
                )
                # gathered chunk -> kT cols [2c0 : 2c0+1024], one DMA per
                # rank slot (AP balancer caps at 3 dims)
                for r in range(2):
                    e = (nc.scalar, nc.sync)[r]
                    e.dma_start(kT[:, :, 2 * c0 + r * 512:2 * c0 + (r + 1) * 512],
                                cc_outk[lc][r])

            # ---- Phase B: local V projection chunks, same exchange ----
            for lc in range(LC):
                c0 = lc * 512
                # scratch tiles covered only by this chunk's own readback
                # (lc0 -> tiles 0:4, lc1 -> tiles 12:16)
                t0 = 0 if lc == 0 else SB - 4
                psvs = banks(f"psv{lc}_")
                if lc == 0:
                    for dt in range(DT):
                        for j4 in range(4):
                            for ec in range(EC):
                                nc.tensor.matmul(psvs[j4 * 2 + ec][:],
                                                 xl_sb[:, dt, c0 + j4 * P:c0 + (j4 + 1) * P],
                                                 wv_sb[:, dt, ec * 512:(ec + 1) * 512],
                                                 start=(dt == 0), stop=(dt == DT - 1))
                    for j4 in range(4):
                        for ec in range(EC):
                            nc.any.tensor_copy(vN[:, t0 + j4, ec * 512:(ec + 1) * 512],
                                               psvs[j4 * 2 + ec][:])
                else:
                    for j4 in range(4):
                        for ec in range(EC):
                            for dt in range(DT):
                                nc.tensor.matmul(psvs[j4 * 2 + ec][:],
                                                 xl_sb[:, dt, c0 + j4 * P:c0 + (j4 + 1) * P],
                                                 wv_sb[:, dt, ec * 512:(ec + 1) * 512],
                                                 start=(dt == 0), stop=(dt == DT - 1))
                            nc.any.tensor_copy(vN[:, t0 + j4, ec * 512:(ec + 1) * 512],
                                               psvs[j4 * 2 + ec][:])
                nc.sync.dma_start(cc_inv[lc][:, 0:2, :], vN[:, t0:t0 + 2, :])
                nc.scalar.dma_start(cc_inv[lc][:, 2:4, :], vN[:, t0 + 2:t0 + 4, :])
                nc.gpsimd.collective_compute(
                    "AllGather", mybir.AluOpType.bypass, replica_groups=GROUPS,
                    ins=[cc_inv[lc][:]], outs=[cc_outv[lc][:]],
                )
                for r in range(2):
                    e = (nc.scalar, nc.sync)[r]
                    e.dma_start(vN[:, 8 * lc + 4 * r:8 * lc + 4 * (r + 1), :],
                                cc_outv[lc][r])

            # ---- Phase C: Q projection (weights long resident) ----
            for g in range(IG):
                psqs = banks(f"psq{g}_")
                for et in range(ET):
                    for dt in range(DT):
                        nc.tensor.matmul(psqs[et][:], wq_sb[:, dt, et * P:(et + 1) * P],
                                         xl_sb[:, dt, g * 512:(g + 1) * 512],
                                         start=(dt == 0), stop=(dt == DT - 1))
                    nc.vector.tensor_scalar_add(qT[:, et, g * 512:(g + 1) * 512],
                                                psqs[et][:], bqk_sb[:, et:et + 1])
            for ec in range(EC):
                pstmp = bigps.tile([P, 512], F32, tag=f"b{ec}", name=f"pstmp{ec}")
                nc.tensor.matmul(pstmp[:], ones_row[:], bv_bf[:, ec * 512:(ec + 1) * 512],
                                 start=True, stop=True)
                nc.any.tensor_copy(bv_bcast[:, ec * 512:(ec + 1) * 512], pstmp[:])

        # ---- Phase D: attention over gather-ordered tiles ----
        with tc.tile_pool(name="sps", bufs=2, space="PSUM") as ps512, \
             tc.tile_pool(name="outps", bufs=2, space="PSUM") as outps, \
             tc.tile_pool(name="rsps", bufs=2, space="PSUM") as rsps, \
             tc.tile_pool(name="attn", bufs=2) as attnp, \
             tc.tile_pool(name="epi", bufs=2) as epip:
            aTs = {}
            for g in range(IG):
                aTs[g] = attnp.tile([P, SB, 512], BF, tag="attn", name=f"aT{g}")
            def scores_tile(g, jt):
                sps = ps512.tile([P, 512], F32, tag="ps512", name="sps")
                for et in range(ET):
                    nc.tensor.matmul(sps[:], kT[:, et, jt * P:(jt + 1) * P],
                                     qT[:, et, g * 512:(g + 1) * 512],
                                     start=(et == 0), stop=(et == ET - 1))
                nc.scalar.activation(aTs[g][:, jt, :], sps[:], AF.Exp,
                                     scale=INV_SQRT_D)
            # first AllGather's tiles for both groups, then the second's
            for g in range(IG):
                for jt in range(HB):
                    scores_tile(g, jt)
            for g in range(IG):
                for jt in range(HB, SB):
                    scores_tile(g, jt)

            for g in range(IG):
                aT = aTs[g]
                invT = epip.tile([P, 8], F32, tag="invT")
                for ib in range(4):
                    rs_ps = rsps.tile([P, 2], F32, tag="rs", name=f"rs{g}_{ib}")
                    last = (g == IG - 1 and ib == 3)
                    out_ps = [outps.tile([P, 512], F32, tag=f"outps{ec}", name=f"out_ps{ec}")
                              for ec in range(EC)]
                    if not last:
                        for jg in range(SB):
                            for ec in range(EC):
                                nc.tensor.matmul(out_ps[ec][:],
                                                 aT[:, jg, ib * P:(ib + 1) * P],
                                                 vN[:, jg, ec * 512:(ec + 1) * 512],
                                                 start=(jg == 0), stop=(jg == SB - 1))
                            nc.tensor.matmul(rs_ps[:, 0:1],
                                             aT[:, jg, ib * P:(ib + 1) * P],
                                             ones_col[:],
                                             start=(jg == 0), stop=(jg == SB - 1))
                    else:
                        for jg in range(SB):
                            nc.tensor.matmul(rs_ps[:, 0:1],
                                             aT[:, jg, ib * P:(ib + 1) * P],
                                             ones_col[:],
                                             start=(jg == 0), stop=(jg == SB - 1))
                        for ec in range(EC):
                            for jg in range(SB):
                                nc.tensor.matmul(out_ps[ec][:],
                                                 aT[:, jg, ib * P:(ib + 1) * P],
                                                 vN[:, jg, ec * 512:(ec + 1) * 512],
                                                 start=(jg == 0), stop=(jg == SB - 1))
                    nc.vector.reciprocal(invT[:, 2 * ib:2 * ib + 1],
                                         rs_ps[:, 0:1])
                    out_sb = epip.tile([P, D], F32, tag="out_sb")
                    r0 = g * 512 + ib * P
                    if not last:
                        for ec in range(EC):
                            nc.vector.tensor_scalar_mul(out_sb[:, ec * 512:(ec + 1) * 512],
                                                        out_ps[ec][:], invT[:, 2 * ib:2 * ib + 1])
                        nc.vector.tensor_add(out_sb[:], out_sb[:], bv_bcast[:])
                        e = (nc.sync, nc.gpsimd, nc.scalar)[ib % 3]
                        e.dma_start(out[r0:r0 + P, :], out_sb[:])
                    else:
                        for ec in range(EC):
                            nc.vector.tensor_scalar_mul(out_sb[:, ec * 512:(ec + 1) * 512],
                                                        out_ps[ec][:], invT[:, 2 * ib:2 * ib + 1])
                            nc.vector.tensor_add(out_sb[:, ec * 512:(ec + 1) * 512],
                                                 out_sb[:, ec * 512:(ec + 1) * 512],
                                                 bv_bcast[:, ec * 512:(ec + 1) * 512])
                            e = (nc.sync, nc.gpsimd)[ec]
                            e.dma_start(out[r0:r0 + P, ec * 512:(ec + 1) * 512],
                                        out_sb[:, ec * 512:(ec + 1) * 512])

    nc.compile()
    return nc


def make_in_maps(x, Wq, bq, Wk, bk, Wv, bv):
    x = np.asarray(x, np.float32)
    wqT = np.asarray(Wq, np.float32).T.astype(BF16)
    wkT = np.asarray(Wk, np.float32).T.astype(BF16)
    wvT = np.asarray(Wv, np.float32).T.astype(BF16)
    bq = np.asarray(bq, np.float32)
    bk = np.asarray(bk, np.float32)
    bqk = np.ascontiguousarray(
        np.concatenate([bq.reshape(8, P).T, bk.reshape(8, P).T], axis=1)
    ).astype(np.float32)
    bv = np.ascontiguousarray(np.asarray(bv, np.float32))
    in_maps = []
    for c in range(NCORES):
        b, h = c // 2, c % 2
        xb = x[b]
        in_maps.append({
            "xlT": xb[h * NQ:(h + 1) * NQ].T.astype(BF16),
            "wqT": wqT, "wkT": wkT, "wvT": wvT,
            "bqk": bqk, "bv": bv,
        })
    return in_maps


def get_nc():
    if "nc" not in _CACHE:
        _CACHE["nc"] = build_nc()
    return _CACHE["nc"]


def kernel(x, Wq, bq, Wk, bk, Wv, bv):
    from concourse.bass_utils import run_bass_kernel_spmd
    nc = get_nc()
    in_maps = make_in_maps(x, Wq, bq, Wk, bk, Wv, bv)
    res = run_bass_kernel_spmd(nc, in_maps, core_ids=list(range(NCORES)))
    out = np.empty((B, S, D), np.float32)
    for c in range(NCORES):
        b, h = c // 2, c % 2
        out[b, h * NQ:(h + 1) * NQ] = res.results[c]["out"]
    return out
